# revision 21
# baseline (speedup 1.0000x reference)
"""CrossProductLayer kernel for Trainium2 (Bass/Tile), 8-core data parallel.

out[b, :] = concat(x[b]**2, x[b], 0.5 * x[b,i]*x[b,j] for i<j) * w

Full inputs:  x [16384, 128] f32, w [8384] f32 -> output [16384, 8384] f32.
Pure batch data parallelism: each of 8 cores computes 2048 rows; w is
pre-scaled (0.5 on the pair block) and pre-broadcast to [128, 8384]
host-side and replicated. Forward only, no collectives.

Per-core kernel: 16 row-tiles of 128 rows. Units = (2 groups of G=8
row-tiles) x (8 column chunks of 1048). Unit tile [128, 8*1048] f32
(33.5 KB/partition, bufs=4 for a deep pipeline window).

Each pair block i (out[:, blk] = x[:,i]*x[:,i+1:]) is one grouped
broadcast tensor_tensor op per unit over [128, 8, w] APs. Within every
chunk the blocks form three contiguous stripes, in column order:
  [ScalarE: widest blocks, per-tile activation ops, capped budget]
  [VectorE: middle blocks]
  [GpSimdE: narrowest tail  (lowest per-op fixed cost)]
The *w pass is ownership-aligned: GpSimdE multiplies its own columns on
its own queue; VectorE multiplies the chunk prefix (head + ScalarE +
VectorE columns) as one contiguous op — so neither W op ever waits on
the other compute engine mid-unit (only VectorE's W waits on ScalarE,
which is issued first and finishes earlier). VectorE issues only
tensor_tensor ops (1-port) so GpSimdE never contends for the shared
SBUF port; the GpSimd broadcast operand is src1 (src0 step-0 innermost
is a slow path). Stores issue at the next loop iteration head.
"""

import numpy as np

B = 16384
NI = 128
NF = NI + NI + (NI * (NI - 1)) // 2  # 8384
NCORES = 8
ROWS = B // NCORES
TILE_P = 128
TILES = ROWS // TILE_P  # 16
PAIRS_OFF = 2 * NI

G = 8
NG = TILES // G  # 2 groups
NCH = 8
CHW = NF // NCH  # 1048

WIDTHS = [NI - 1 - i for i in range(NI - 1)]
STARTS = []
_off = PAIRS_OFF
for _w in WIDTHS:
    STARTS.append(_off)
    _off += _w
assert _off == NF

# fitted per-op costs (ns) per unit (G=8 rows)
ACT_OP = lambda w: 371.0 + 0.83 * w  # per tile-op => G per unit
DVE_OP = lambda w: 620.0 + 1.09 * G * w
GP_OP = lambda w: 390.0 + 1.93 * G * w
DVE_WCOL = 1.09 * G
GP_WCOL = 1.93 * G

ACT_BUDGET = 155e3  # ns, total


def _pieces(ch):
    lo, hi = ch * CHW, (ch + 1) * CHW
    out = []
    for i in range(NI - 1):
        s, w = STARTS[i], WIDTHS[i]
        a, b = max(s, lo), min(s + w, hi)
        if a < b:
            out.append((i, a, b - a))
    return out


def _plan():
    """Per chunk: contiguous [ACT prefix | DVE middle | GP tail]; the
    DVE/GP boundary balances per-chunk loads incl. the owned *w cols."""
    plan = []
    act_per_chunk = ACT_BUDGET / NCH
    for ch in range(NCH):
        ps = _pieces(ch)  # in block order; widths descending
        a_load = 16 * (371 + 128) / 1.2 / NCH if ch == 0 else 0.0
        na = 0
        for i, cs, w in ps:
            c = 16 * ACT_OP(w)
            if a_load + c > act_per_chunk:
                break
            a_load += c
            na += 1
        head = PAIRS_OFF if ch == 0 else 0
        a_cols = head + sum(w for _, _, w in ps[:na])
        best = None
        for nb in range(na, len(ps) + 1):
            mid = ps[na:nb]
            tail = ps[nb:]
            d_cols = sum(w for _, _, w in mid)
            g_cols = sum(w for _, _, w in tail)
            d = sum(2 * DVE_OP(w) for _, _, w in mid)
            d += 2 * DVE_WCOL * (a_cols + d_cols)
            gl = sum(2 * GP_OP(w) for _, _, w in tail)
            gl += 2 * GP_WCOL * g_cols
            mk = max(a_load, d, gl)
            if best is None or mk < best[0]:
                best = (mk, nb, d, gl)
        _, nb, d_load, g_load = best
        plan.append(
            {
                "ch": ch,
                "pieces": ps,
                "na": na,
                "nb": nb,
                "loads": (a_load, d_load, g_load),
            }
        )
    return plan


PLAN = _plan()

_CACHE = {}


def _build_nc():
    import os

    os.environ["TILE_EXHAUSTIVE_MEMORY_SHARE_CHECK"] = "1"
    from concourse import bacc
    import concourse.mybir as mybir
    from concourse.tile import TileContext

    f32 = mybir.dt.float32
    nc = bacc.Bacc(
        "TRN2", target_bir_lowering=False, debug=False, num_devices=NCORES
    )
    x_d = nc.dram_tensor("x", [ROWS, NI], f32, kind="ExternalInput")
    w_d = nc.dram_tensor("w", [NI, NF], f32, kind="ExternalInput")
    o_d = nc.dram_tensor("out", [ROWS, NF], f32, kind="ExternalOutput")

    x_hbm3 = x_d.rearrange("(t p) c -> p t c", t=TILES)
    o_hbm3 = o_d.rearrange("(t p) c -> p t c", t=TILES)

    units = [(g, ch) for g in range(NG) for ch in range(NCH)]

    with TileContext(nc) as tc:
        with (
            tc.tile_pool(name="xp", bufs=1) as xp,
            tc.tile_pool(name="wp", bufs=1) as wp,
            tc.tile_pool(name="pp", bufs=4) as pp,
        ):
            x_all = xp.tile([TILE_P, TILES * NI], f32)
            x3 = x_all[:].rearrange("p (t c) -> p t c", t=TILES)
            nc.sync.dma_start(out=x3, in_=x_hbm3)
            w_t = wp.tile([NI, NF], f32)
            nc.sync.dma_start(out=w_t[:], in_=w_d[:])

            pending = []

            def flush():
                while pending:
                    pan3, t0, t1, lo = pending.pop(0)
                    nc.sync.dma_start(
                        out=o_hbm3[:, t0:t1, lo : lo + CHW], in_=pan3
                    )

            for k, (g, ch) in enumerate(units):
                flush()
                info = PLAN[ch]
                ps, na, nb = info["pieces"], info["na"], info["nb"]
                t0, t1 = g * G, (g + 1) * G
                lo = ch * CHW
                pan = pp.tile(
                    [TILE_P, G * CHW], f32, name=f"pan{g}{ch}", tag="pan"
                )
                pan3 = pan[:].rearrange("p (t c) -> p t c", t=G)
                if ch == 0:
                    nc.sync.dma_start(
                        out=pan3[:, :, NI : 2 * NI], in_=x_hbm3[:, t0:t1]
                    )
                    nc.scalar.square(pan3[:, :, 0:NI], x3[:, t0:t1])

                def emit(piece, eng):
                    i, cs, w = piece
                    c = cs - lo
                    j0 = i + 1 + (cs - STARTS[i])
                    src = x3[:, t0:t1, j0 : j0 + w]
                    dst = pan3[:, :, c : c + w]
                    if eng == "A":
                        for t in range(G):
                            nc.scalar.mul(
                                dst[:, t], src[:, t], x3[:, t0 + t, i : i + 1]
                            )
                    else:
                        bc = x3[:, t0:t1, i : i + 1].broadcast_to(
                            [TILE_P, G, w]
                        )
                        if eng == "G":
                            nc.gpsimd.tensor_mul(dst, src, bc)
                        else:
                            nc.vector.tensor_mul(dst, bc, src)

                for p in ps[:na]:
                    emit(p, "A")
                for p in ps[nb:]:
                    emit(p, "G")
                for p in ps[na:nb]:
                    emit(p, "D")
                # ownership-aligned *w: GP covers its tail, DVE the rest
                g_start = ps[nb][1] - lo if nb < len(ps) else CHW
                wsl = w_t[:, None, lo : lo + CHW]
                if g_start < CHW:
                    nc.gpsimd.tensor_mul(
                        pan3[:, :, g_start:CHW],
                        pan3[:, :, g_start:CHW],
                        wsl[:, :, g_start:CHW].broadcast_to(
                            [TILE_P, G, CHW - g_start]
                        ),
                    )
                if g_start > 0:
                    nc.vector.tensor_mul(
                        pan3[:, :, 0:g_start],
                        pan3[:, :, 0:g_start],
                        wsl[:, :, 0:g_start].broadcast_to(
                            [TILE_P, G, g_start]
                        ),
                    )
                pending.append((pan3, t0, t1, lo))
            flush()
    nc.compile()
    return nc


def _get_nc():
    if "nc" not in _CACHE:
        _CACHE["nc"] = _build_nc()
    return _CACHE["nc"]


def _prep_in_maps(x, w):
    x = np.ascontiguousarray(np.asarray(x, dtype=np.float32))
    w = np.asarray(w, dtype=np.float32)
    w_scaled = w.copy()
    w_scaled[PAIRS_OFF:] *= np.float32(0.5)
    w_b = np.ascontiguousarray(np.broadcast_to(w_scaled[None, :], (NI, NF)))
    return [
        {"x": np.ascontiguousarray(x[c * ROWS : (c + 1) * ROWS]), "w": w_b}
        for c in range(NCORES)
    ]


def _run(x, w, trace=False, tmpdir=None):
    from concourse.bass_utils import run_bass_kernel_spmd

    nc = _get_nc()
    in_maps = _prep_in_maps(x, w)
    res = run_bass_kernel_spmd(
        nc, in_maps, list(range(NCORES)), trace=trace, tmpdir=tmpdir
    )
    out = np.concatenate([res.results[c]["out"] for c in range(NCORES)], axis=0)
    return out, res


def kernel(**inputs):
    out, _ = _run(inputs["x"], inputs["w"])
    return out


if __name__ == "__main__":
    for p in PLAN:
        a, dl, gl = p["loads"]
        print(
            f"chunk {p['ch']}: A/D/G pieces {p['na']}/"
            f"{p['nb']-p['na']}/{len(p['pieces'])-p['nb']} "
            f"loads A={a/1e3:6.1f} D={dl/1e3:6.1f} G={gl/1e3:6.1f} us"
        )
    tot = [sum(p["loads"][j] for p in PLAN) / 1e3 for j in range(3)]
    print(f"totals A={tot[0]:.0f} D={tot[1]:.0f} G={tot[2]:.0f} us")


# revision 22
# speedup vs baseline: 1.0725x; 1.0725x over previous
"""CrossProductLayer kernel for Trainium2 (Bass/Tile), 8-core data parallel.

out[b, :] = concat(x[b]**2, x[b], 0.5 * x[b,i]*x[b,j] for i<j) * w

Full inputs:  x [16384, 128] f32, w [8384] f32.
Full output:  [16384, 8384] f32.

Sharding: pure data parallel on the batch dim — each of the 8 cores gets
2048 rows of x; w (pre-scaled and pre-broadcast to [128, 8384] on host) is
replicated. No collectives needed (forward only).

Per-core device kernel (16 row-tiles of 128 batch rows):
  - squares  -> ScalarE (Square activation)
  - singles  -> DMA'd straight from HBM into the output tile
  - pairs    -> per-i blocks out[:, blk_i] = x[:, i] * x[:, i+1:]:
               wide blocks (i < K_ACT) on ScalarE via activation scale,
               the rest on VectorE tensor_scalar (fp32 2x mode; odd widths
               padded by one column which the next block overwrites)
  - *w pass  -> one full-width VectorE tensor_tensor multiply
  - store    -> one 4.3 MB HWDGE DMA per tile
"""

import numpy as np

B = 16384
NI = 128
NF = NI + NI + (NI * (NI - 1)) // 2  # 8384
NCORES = 8
ROWS = B // NCORES  # 2048
TILE_P = 128
TILES = ROWS // TILE_P  # 16
PAIRS_OFF = 2 * NI  # 256
K_ACT = 53  # pair blocks 0..K_ACT-1 run on ScalarE, the rest on VectorE

_CACHE = {}


def _build_nc():
    import os

    # precise (unbounded) overlap tracking: the padded TS blocks and the
    # half-tile *w passes need byte-range-accurate deps, not the
    # conservative fallback past 100 pairwise checks
    os.environ["TILE_EXHAUSTIVE_MEMORY_SHARE_CHECK"] = "1"
    from concourse import bacc
    import concourse.mybir as mybir
    from concourse.tile import TileContext

    f32 = mybir.dt.float32
    nc = bacc.Bacc(
        "TRN2",
        target_bir_lowering=False,
        debug=False,
        num_devices=NCORES,
    )
    x_d = nc.dram_tensor("x", [ROWS, NI], f32, kind="ExternalInput")
    w_d = nc.dram_tensor("w", [NI, NF], f32, kind="ExternalInput")
    o_d = nc.dram_tensor("out", [ROWS, NF], f32, kind="ExternalOutput")

    with TileContext(nc) as tc:
        with (
            tc.tile_pool(name="wp", bufs=1) as wp,
            tc.tile_pool(name="xp", bufs=4) as xp,
            tc.tile_pool(name="op", bufs=4) as op,
        ):
            w_t = wp.tile([NI, NF], f32)
            nc.sync.dma_start(out=w_t[:], in_=w_d[:])
            for t in range(TILES):
                r0 = t * TILE_P
                x_t = xp.tile([TILE_P, NI + 2], f32)
                nc.sync.dma_start(out=x_t[:, 0:NI], in_=x_d[r0 : r0 + TILE_P])
                # output tile; 16 spare cols so the last padded pair block
                # can spill one column past NF
                o_t = op.tile([TILE_P, NF + 16], f32)
                # singles block [NI:2*NI) comes straight from HBM
                nc.sync.dma_start(out=o_t[:, NI : 2 * NI], in_=x_d[r0 : r0 + TILE_P])
                # squares block [0:NI)
                nc.scalar.square(o_t[:, 0:NI], x_t[:, 0:NI])
                off = PAIRS_OFF
                for i in range(NI - 1):
                    wdt = NI - 1 - i
                    sc = x_t[:, i : i + 1]
                    if i < K_ACT:
                        nc.scalar.mul(
                            o_t[:, off : off + wdt], x_t[:, i + 1 : i + 1 + wdt], sc
                        )
                    else:
                        # pad odd widths to even for the DVE fp32 2x mode;
                        # the padded column is overwritten by block i+1
                        wpad = wdt + (wdt & 1)
                        nc.vector.tensor_scalar_mul(
                            o_t[:, off : off + wpad],
                            x_t[:, i + 1 : i + 1 + wpad],
                            sc,
                        )
                    off += wdt
                # the *w pass and store in two halves: the first half's
                # store can start while the second half is still being
                # multiplied (16.8 KB HBM rows stay at full DMA rate)
                H = NF // 2
                nc.vector.tensor_mul(o_t[:, 0:H], o_t[:, 0:H], w_t[:, 0:H])
                nc.sync.dma_start(
                    out=o_d[r0 : r0 + TILE_P, 0:H], in_=o_t[:, 0:H]
                )
                nc.vector.tensor_mul(o_t[:, H:NF], o_t[:, H:NF], w_t[:, H:NF])
                nc.sync.dma_start(
                    out=o_d[r0 : r0 + TILE_P, H:NF], in_=o_t[:, H:NF]
                )
    nc.compile()
    return nc


def _get_nc():
    if "nc" not in _CACHE:
        _CACHE["nc"] = _build_nc()
    return _CACHE["nc"]


def _prep_in_maps(x, w):
    x = np.ascontiguousarray(np.asarray(x, dtype=np.float32))
    w = np.asarray(w, dtype=np.float32)
    w_scaled = w.copy()
    w_scaled[PAIRS_OFF:] *= np.float32(0.5)
    w_b = np.ascontiguousarray(np.broadcast_to(w_scaled[None, :], (NI, NF)))
    return [
        {"x": np.ascontiguousarray(x[c * ROWS : (c + 1) * ROWS]), "w": w_b}
        for c in range(NCORES)
    ]


def _run(x, w, trace=False, tmpdir=None):
    from concourse.bass_utils import run_bass_kernel_spmd

    nc = _get_nc()
    in_maps = _prep_in_maps(x, w)
    res = run_bass_kernel_spmd(
        nc, in_maps, list(range(NCORES)), trace=trace, tmpdir=tmpdir
    )
    out = np.concatenate([res.results[c]["out"] for c in range(NCORES)], axis=0)
    return out, res


def kernel(**inputs):
    out, _ = _run(inputs["x"], inputs["w"])
    return out
